# revision 4
# baseline (speedup 1.0000x reference)
"""Mixtral MoE layer (8 experts, top-2, H=2048, I=7168, T=8192) on 8 trn2 NeuronCores.

Inter-sliced data parallel ("tensor parallel over I"): core c owns rows
[c*896, (c+1)*896) of EVERY expert's w1/w3 (and the matching columns of w2).
All cores process ALL routed token-expert pairs over their inter slice and
emit partial outputs; the host sums the 8 partials in f32 (exact balance by
construction — no padding to the max-loaded expert, which a pure
expert-parallel split pays on every core).

The router (gate matmul + softmax + top-2 + renormalize) runs on host CPU with
the exact op sequence of the reference; tokens are gathered per expert on host
(the "all-to-all dispatch") into one column-packed activation tensor shared by
all cores, and the host scatter-adds the weighted summed partials back (the
"all-to-all combine").

Device-side layout avoids all on-chip transposes. Per token block b (one
expert e_b, tb<=512 columns, structure identical on all cores):
  phase A:  Gt[j][i, c] = silu(W1t[j].T @ Xt) * (W3t[j].T @ Xt)  (j in 7
            slice chunks of 128 inter rows; 16-deep PSUM chains over H)
  phase B:  Yt[h, c] = sum_u W2t[h, u].T @ Gt[u]                 (h-outer,
            7-deep PSUM chain per h)
with Xt = X.T etc., all pre-tiled on host for contiguous DMA runs.

DMA queue plan (2 HWDGE queues + gpsimd SWDGE), per block ~15 MB:
  sync   : x block + w1 slice groups
  scalar : w3 slice groups
  gpsimd : w2 h-tiles + output writes
"""

import math

import numpy as np
import ml_dtypes

import concourse.bass as bass
import concourse.mybir as mybir
import concourse.tile as tile
from concourse.bass_utils import run_bass_kernel_spmd

H = 2048          # hidden dim
I = 7168          # intermediate dim
E = 8             # experts = cores
TOPK = 2
HJ = H // 128     # 16 hidden chunks of 128
IS = I // E       # 896 inter rows per core slice
JS = IS // 128    # 7 slice chunks of 128
TBMAX = 512       # token block (matmul free dim, one PSUM bank)

BF16 = mybir.dt.bfloat16
F32 = mybir.dt.float32

last_exec_time_ns = None  # set when BASS_MOE_TRACE=1
last_results = None


def _install_axon_hooks_shim():
    """This image lacks antenv.axon_hooks (needed by run_bass_kernel_spmd
    trace=True). Provide it, with the NTFF profile hook driven via ctypes
    into the injected axon .so (mirrors trn_agent_boot._ntff_profile_via_ctypes)."""
    import sys

    try:
        import antenv.axon_hooks  # noqa: F401

        return
    except ImportError:
        pass
    import contextlib
    import ctypes
    import types

    hook = None
    so_path = "/opt/axon/libaxon_pjrt.so"
    try:
        lib = ctypes.CDLL(so_path)
        if hasattr(lib, "axon_start_nrt_profile"):
            lib.axon_start_nrt_profile.argtypes = [
                ctypes.POINTER(ctypes.c_int64),
                ctypes.c_size_t,
            ]
            lib.axon_start_nrt_profile.restype = ctypes.c_int64
            lib.axon_stop_nrt_profile.argtypes = [ctypes.c_char_p]
            lib.axon_stop_nrt_profile.restype = ctypes.c_int64

            @contextlib.contextmanager
            def _hook(output_dir, device_ids):
                import jax

                jax.devices()
                if device_ids:
                    ids = (ctypes.c_int64 * len(device_ids))(*device_ids)
                    rc = lib.axon_start_nrt_profile(ids, len(device_ids))
                else:
                    rc = lib.axon_start_nrt_profile(None, 0)
                if rc != 0:
                    raise RuntimeError(f"axon_start_nrt_profile rc={rc}")
                try:
                    yield
                finally:
                    n = lib.axon_stop_nrt_profile(str(output_dir).encode())
                    print(f"ntff profile: {n} file(s) -> {output_dir}", flush=True)

            hook = _hook
    except OSError:
        pass

    mod = types.ModuleType("antenv.axon_hooks")
    mod._hook = hook
    mod.get_axon_ntff_profile_hook = lambda: mod._hook
    mod.set_axon_ntff_profile_hook = lambda h: setattr(mod, "_hook", h)
    sys.modules["antenv.axon_hooks"] = mod


_install_axon_hooks_shim()


def legalize_single_wait(nc):
    """This walrus rejects >1 sem wait per instruction: hoist extras onto
    preceding NoOps on the same engine (per-engine program order preserved)."""
    n_split = 0
    for fn in nc.m.functions:
        for blk in fn.blocks:
            new = []
            for inst in blk.instructions:
                si = inst.sync_info
                if si is not None and si.on_wait and len(si.on_wait) > 1:
                    waits = list(si.on_wait)
                    for i, w in enumerate(waits[:-1]):
                        nop = mybir.InstNoOp(name=f"{inst.name}-w{i}", ins=[], outs=[])
                        nop.engine = inst.engine
                        nop.sync_info = mybir.SyncInfo(on_wait=[w], on_update=[])
                        new.append(nop)
                        n_split += 1
                    inst.sync_info = mybir.SyncInfo(
                        on_wait=[waits[-1]], on_update=list(si.on_update)
                    )
                new.append(inst)
            blk.instructions[:] = new
    return n_split


def _split_blocks(n):
    """Near-equal multiple-of-4 block sizes <= TBMAX covering ceil4(n) cols."""
    n4 = (n + 3) // 4 * 4
    nb = max(1, math.ceil(n4 / TBMAX))
    base = (n4 // nb) // 4 * 4
    rem = (n4 - base * nb) // 4
    tbs = [base + (4 if i < rem else 0) for i in range(nb)]
    assert sum(tbs) == n4 and all(0 < t <= TBMAX for t in tbs)
    return tbs


_programs = {}


def _build_program(blocks):
    """One SPMD program: every core runs the same block schedule over its own
    inter slice. blocks = tuple of (expert_id, tb)."""
    if blocks in _programs:
        return _programs[blocks]

    C = sum(tb for _, tb in blocks)
    nc = bass.Bass("TRN2", target_bir_lowering=False, debug=False, num_devices=E)
    xt = nc.declare_dram_parameter("xt", [HJ, 128, C], BF16, isOutput=False)
    # per-expert slice weights, pre-tiled: w1/w3 [E, JS, 128(h-in-chunk),
    # HJ, 128(inter)] ; w2 [E, HJ, 128(i-in-chunk), JS*128(h-col-major u,m)]
    w1 = nc.declare_dram_parameter("w1", [E, JS, 128, HJ, 128], BF16, isOutput=False)
    w3 = nc.declare_dram_parameter("w3", [E, JS, 128, HJ, 128], BF16, isOutput=False)
    w2 = nc.declare_dram_parameter("w2", [E, HJ, 128, JS * 128], BF16, isOutput=False)
    yt = nc.declare_dram_parameter("yt", [HJ, 128, C], BF16, isOutput=True)

    n_blocks = len(blocks)
    with tile.TileContext(nc) as tc:
        with (
            tc.tile_pool(name="xp", bufs=2) as xp,
            tc.tile_pool(name="w1p", bufs=9) as w1p,
            tc.tile_pool(name="w3p", bufs=9) as w3p,
            tc.tile_pool(name="w2p", bufs=6) as w2p,
            tc.tile_pool(name="gtp", bufs=2 * JS) as gtp,
            tc.tile_pool(name="sip", bufs=3) as sip,
            tc.tile_pool(name="otp", bufs=16) as otp,
            tc.tile_pool(name="pga", bufs=2, space="PSUM") as pga,
            tc.tile_pool(name="pob", bufs=4, space="PSUM") as pob,
        ):
            c0 = 0
            for cb, (eb, tb) in enumerate(blocks):
                xsb = xp.tile([128, HJ, tb], BF16, tag="xsb")
                w1sbs = {}
                w3sbs = {}
                if cb == 0:
                    # startup: interleave x and the first two w1 groups in
                    # fine-grained k-order on the sync queue so the first
                    # chains unblock piecewise; w3 j=0/1 follow on sync (its
                    # chain runs second), the rest ride the regular queues
                    w1sbs[0] = w1p.tile([128, HJ, 128], BF16, tag="w1sb", name="w1sb")
                    w3sbs[0] = w3p.tile([128, HJ, 128], BF16, tag="w3sb", name="w3sb")
                    w1sbs[1] = w1p.tile([128, HJ, 128], BF16, tag="w1sb", name="w1sb")
                    w3sbs[1] = w3p.tile([128, HJ, 128], BF16, tag="w3sb", name="w3sb")
                    for kk in range(0, HJ, 4):
                        nc.sync.dma_start(
                            out=xsb[:, kk : kk + 4, :],
                            in_=xt[kk : kk + 4, :, c0 : c0 + tb].rearrange(
                                "j p c -> p j c"
                            ),
                        )
                        nc.sync.dma_start(
                            out=w1sbs[0][:, kk : kk + 4, :],
                            in_=w1[eb, 0, :, kk : kk + 4, :],
                        )
                    for kk in range(0, HJ, 4):
                        nc.sync.dma_start(
                            out=w3sbs[0][:, kk : kk + 4, :],
                            in_=w3[eb, 0, :, kk : kk + 4, :],
                        )
                    for kk in range(0, HJ, 8):
                        nc.sync.dma_start(
                            out=w1sbs[1][:, kk : kk + 8, :],
                            in_=w1[eb, 1, :, kk : kk + 8, :],
                        )
                    for kk in range(0, HJ, 8):
                        nc.sync.dma_start(
                            out=w3sbs[1][:, kk : kk + 8, :],
                            in_=w3[eb, 1, :, kk : kk + 8, :],
                        )
                else:
                    nc.sync.dma_start(
                        out=xsb[:, :, :],
                        in_=xt[:, :, c0 : c0 + tb].rearrange("j p c -> p j c"),
                    )

                for j in range(JS):
                    if j not in w1sbs:
                        w1sbs[j] = w1p.tile([128, HJ, 128], BF16, tag="w1sb", name="w1sb")
                        nc.sync.dma_start(out=w1sbs[j][:, :, :], in_=w1[eb, j])
                        w3sbs[j] = w3p.tile([128, HJ, 128], BF16, tag="w3sb", name="w3sb")
                        w3eng = nc.sync if (cb == 0 and j == 2) else nc.scalar
                        w3eng.dma_start(out=w3sbs[j][:, :, :], in_=w3[eb, j])

                w2sbs = {}

                def issue_w2(h):
                    if h in w2sbs or h >= HJ:
                        return
                    t = w2p.tile([128, JS, 128], BF16, tag="w2sb", name="w2sb")
                    nc.gpsimd.dma_start(out=t[:, :, :], in_=w2[eb, h])
                    w2sbs[h] = t

                # ---- phase A: Gt[j][i, c] over the 896-row inter slice ----
                gts = []
                for j in range(JS):
                    # stagger w2 h-tile loads through phase A; on block 0
                    # start later so SWDGE doesn't steal HBM from the fill
                    if cb == 0:
                        w2_until = 0 if j < 3 else ((j - 2) * HJ) // 4
                    else:
                        w2_until = ((j + 1) * HJ + JS - 1) // JS
                    for h in range(len(w2sbs), w2_until):
                        issue_w2(h)
                    pg1 = pga.tile([128, tb], F32, tag="pg1")
                    pg3 = pga.tile([128, tb], F32, tag="pg3")
                    for k in range(HJ):
                        nc.tensor.matmul(
                            pg1[:, :],
                            lhsT=w1sbs[j][:, k, :],
                            rhs=xsb[:, k, :],
                            start=(k == 0),
                            stop=(k == HJ - 1),
                        )
                    for k in range(HJ):
                        nc.tensor.matmul(
                            pg3[:, :],
                            lhsT=w3sbs[j][:, k, :],
                            rhs=xsb[:, k, :],
                            start=(k == 0),
                            stop=(k == HJ - 1),
                        )
                    ssb = sip.tile([128, tb], F32, tag="ssb")
                    nc.scalar.activation(
                        ssb[:, :], pg1[:, :], mybir.ActivationFunctionType.Silu
                    )
                    gt = gtp.tile([128, tb], BF16, tag="gt")
                    nc.vector.tensor_mul(gt[:, :], pg3[:, :], ssb[:, :])
                    gts.append(gt)
                for h in range(HJ):
                    issue_w2(h)

                # ---- phase B: Yt[h, c] = sum_u W2t[h,u].T @ Gt[u], h-outer ----
                last_cb = cb == n_blocks - 1
                for h in range(HJ):
                    po = pob.tile([128, tb], F32, tag="po")
                    for u in range(JS):
                        nc.tensor.matmul(
                            po[:, :],
                            lhsT=w2sbs[h][:, u, :],
                            rhs=gts[u][:, :],
                            start=(u == 0),
                            stop=(u == JS - 1),
                        )
                    ot = otp.tile([128, tb], BF16, tag="ot")
                    nc.vector.tensor_copy(ot[:, :], po[:, :])
                    if last_cb:
                        # final block: ship outputs on the idle HWDGE queues
                        # (sub-us fixed cost) so the drain tail is short
                        eng = nc.sync if h % 2 == 0 else nc.scalar
                        eng.dma_start(out=yt[h, :, c0 : c0 + tb], in_=ot[:, :])
                    else:
                        nc.gpsimd.dma_start(out=yt[h, :, c0 : c0 + tb], in_=ot[:, :])
                c0 += tb

    legalize_single_wait(nc)
    _programs[blocks] = nc
    return nc


def _routing(x, gate_weight):
    """Replicate the reference router bitwise-closely: jax on CPU, same ops."""
    import jax
    import jax.numpy as jnp

    cpu = jax.devices("cpu")[0]
    with jax.default_device(cpu):
        router_logits = jnp.asarray(x) @ jnp.asarray(gate_weight).T
        probs = jax.nn.softmax(router_logits.astype(jnp.float32), axis=-1)
        top_w, top_idx = jax.lax.top_k(probs, TOPK)
        top_w = top_w / jnp.sum(top_w, axis=-1, keepdims=True)
        top_w = top_w.astype(x.dtype)
        return np.asarray(top_w), np.asarray(top_idx)


def kernel(hidden_states, gate_weight, w1_weight, w3_weight, w2_weight):
    import os

    x = np.asarray(hidden_states, dtype=np.float32)
    T = x.shape[0]
    top_w, top_idx = _routing(x, np.asarray(gate_weight, dtype=np.float32))

    tok_ids = []
    tok_w = []
    for e in range(E):
        rows, cols = np.nonzero(top_idx == e)
        tok_ids.append(rows)
        tok_w.append(top_w[rows, cols].astype(np.float32))

    # global block schedule, shared by all cores (compile-time constants)
    blocks = []
    expert_off = []
    off = 0
    for e in range(E):
        expert_off.append(off)
        tbs = _split_blocks(len(tok_ids[e]))
        blocks.extend((e, tb) for tb in tbs)
        off += sum(tbs)
    blocks = tuple(blocks)
    C = off

    bf16 = ml_dtypes.bfloat16

    # column-packed activations, shared by every core
    xg = np.zeros((C, H), dtype=bf16)
    for e in range(E):
        xg[expert_off[e] : expert_off[e] + len(tok_ids[e])] = x[tok_ids[e]]
    xtile = np.ascontiguousarray(xg.T).reshape(HJ, 128, C)

    # per-core inter-slice weights, one global reorder each:
    # W1T[c, e, j, p, k, m] = w1[e, c*896 + j*128 + m, k*128 + p]
    w1b = np.asarray(w1_weight, dtype=bf16)
    w3b = np.asarray(w3_weight, dtype=bf16)
    w2b = np.asarray(w2_weight, dtype=bf16)
    w1t = np.ascontiguousarray(
        w1b.reshape(E, E, JS, 128, HJ, 128).transpose(1, 0, 2, 5, 4, 3)
    )
    w3t = np.ascontiguousarray(
        w3b.reshape(E, E, JS, 128, HJ, 128).transpose(1, 0, 2, 5, 4, 3)
    )
    # W2T[c, e, h, p, u, m] = w2[e, h*128 + m, c*896 + u*128 + p]
    w2t = np.ascontiguousarray(
        w2b.reshape(E, HJ, 128, E, JS, 128).transpose(3, 0, 1, 5, 4, 2)
    ).reshape(E, E, HJ, 128, JS * 128)

    in_maps = [
        {"xt": xtile, "w1": w1t[c], "w3": w3t[c], "w2": w2t[c]} for c in range(E)
    ]

    nc = _build_program(blocks)
    trace = os.environ.get("BASS_MOE_TRACE", "") == "1"
    res = None
    if trace:
        import concourse.bass_utils as bu

        orig_upload = bu.upload_artifacts
        bu.upload_artifacts = lambda tmpdir: f"local://{tmpdir}"
        tdir = os.environ.get("BASS_MOE_TRACE_DIR") or None
        try:
            res = run_bass_kernel_spmd(
                nc, in_maps, list(range(E)), trace=True, tmpdir=tdir
            )
        except Exception as exc:
            print(f"trace path failed ({type(exc).__name__}: {exc}); rerunning untraced", flush=True)
            res = None
        finally:
            bu.upload_artifacts = orig_upload
    if res is None:
        res = run_bass_kernel_spmd(nc, in_maps, list(range(E)))
    global last_exec_time_ns, last_results
    last_exec_time_ns = res.exec_time_ns
    last_results = res

    # combine: sum the 8 inter-slice partials in f32, then weighted scatter
    ysum = np.zeros((H, C), dtype=np.float32)
    for c in range(E):
        ysum += np.asarray(res.results[c]["yt"], dtype=np.float32).reshape(H, C)
    out = np.zeros((T, H), dtype=np.float32)
    for e in range(E):
        n_e = len(tok_ids[e])
        o = expert_off[e]
        out[tok_ids[e]] += tok_w[e][:, None] * ysum[:, o : o + n_e].T
    return out


# revision 6
# speedup vs baseline: 1.0261x; 1.0261x over previous
"""Mixtral MoE layer (8 experts, top-2, H=2048, I=7168, T=8192) on 8 trn2 NeuronCores.

Inter-sliced data parallel ("tensor parallel over I"): core c owns rows
[c*896, (c+1)*896) of EVERY expert's w1/w3 (and the matching columns of w2).
All cores process ALL routed token-expert pairs over their inter slice and
emit partial outputs; the host sums the 8 partials in f32 (exact balance by
construction — no padding to the max-loaded expert, which a pure
expert-parallel split pays on every core).

The router (gate matmul + softmax + top-2 + renormalize) runs on host CPU with
the exact op sequence of the reference; tokens are gathered per expert on host
(the "all-to-all dispatch") into one column-packed activation tensor shared by
all cores, and the host scatter-adds the weighted summed partials back (the
"all-to-all combine").

Device-side layout avoids all on-chip transposes. Per token block b (one
expert e_b, tb<=512 columns, structure identical on all cores):
  phase A:  Gt[j][i, c] = silu(W1t[j].T @ Xt) * (W3t[j].T @ Xt)  (j in 7
            slice chunks of 128 inter rows; 16-deep PSUM chains over H)
  phase B:  Yt[h, c] = sum_u W2t[h, u].T @ Gt[u]                 (h-outer,
            7-deep PSUM chain per h)
with Xt = X.T etc., all pre-tiled on host for contiguous DMA runs.

DMA queue plan (2 HWDGE queues + gpsimd SWDGE), per block ~15 MB:
  sync   : x block + w1 slice groups
  scalar : w3 slice groups
  gpsimd : w2 h-tiles + output writes
"""

import math

import numpy as np
import ml_dtypes

import concourse.bass as bass
import concourse.mybir as mybir
import concourse.tile as tile
from concourse.bass_utils import run_bass_kernel_spmd

H = 2048          # hidden dim
I = 7168          # intermediate dim
E = 8             # experts = cores
TOPK = 2
HJ = H // 128     # 16 hidden chunks of 128
IS = I // E       # 896 inter rows per core slice
JS = IS // 128    # 7 slice chunks of 128
TBMAX = 512       # token block (matmul free dim, one PSUM bank)

BF16 = mybir.dt.bfloat16
F32 = mybir.dt.float32

last_exec_time_ns = None  # set when BASS_MOE_TRACE=1
last_results = None


def _install_axon_hooks_shim():
    """This image lacks antenv.axon_hooks (needed by run_bass_kernel_spmd
    trace=True). Provide it, with the NTFF profile hook driven via ctypes
    into the injected axon .so (mirrors trn_agent_boot._ntff_profile_via_ctypes)."""
    import sys

    try:
        import antenv.axon_hooks  # noqa: F401

        return
    except ImportError:
        pass
    import contextlib
    import ctypes
    import types

    hook = None
    so_path = "/opt/axon/libaxon_pjrt.so"
    try:
        lib = ctypes.CDLL(so_path)
        if hasattr(lib, "axon_start_nrt_profile"):
            lib.axon_start_nrt_profile.argtypes = [
                ctypes.POINTER(ctypes.c_int64),
                ctypes.c_size_t,
            ]
            lib.axon_start_nrt_profile.restype = ctypes.c_int64
            lib.axon_stop_nrt_profile.argtypes = [ctypes.c_char_p]
            lib.axon_stop_nrt_profile.restype = ctypes.c_int64

            @contextlib.contextmanager
            def _hook(output_dir, device_ids):
                import jax

                jax.devices()
                if device_ids:
                    ids = (ctypes.c_int64 * len(device_ids))(*device_ids)
                    rc = lib.axon_start_nrt_profile(ids, len(device_ids))
                else:
                    rc = lib.axon_start_nrt_profile(None, 0)
                if rc != 0:
                    raise RuntimeError(f"axon_start_nrt_profile rc={rc}")
                try:
                    yield
                finally:
                    n = lib.axon_stop_nrt_profile(str(output_dir).encode())
                    print(f"ntff profile: {n} file(s) -> {output_dir}", flush=True)

            hook = _hook
    except OSError:
        pass

    mod = types.ModuleType("antenv.axon_hooks")
    mod._hook = hook
    mod.get_axon_ntff_profile_hook = lambda: mod._hook
    mod.set_axon_ntff_profile_hook = lambda h: setattr(mod, "_hook", h)
    sys.modules["antenv.axon_hooks"] = mod


_install_axon_hooks_shim()


def legalize_single_wait(nc):
    """This walrus rejects >1 sem wait per instruction: hoist extras onto
    preceding NoOps on the same engine (per-engine program order preserved)."""
    n_split = 0
    for fn in nc.m.functions:
        for blk in fn.blocks:
            new = []
            for inst in blk.instructions:
                si = inst.sync_info
                if si is not None and si.on_wait and len(si.on_wait) > 1:
                    waits = list(si.on_wait)
                    for i, w in enumerate(waits[:-1]):
                        nop = mybir.InstNoOp(name=f"{inst.name}-w{i}", ins=[], outs=[])
                        nop.engine = inst.engine
                        nop.sync_info = mybir.SyncInfo(on_wait=[w], on_update=[])
                        new.append(nop)
                        n_split += 1
                    inst.sync_info = mybir.SyncInfo(
                        on_wait=[waits[-1]], on_update=list(si.on_update)
                    )
                new.append(inst)
            blk.instructions[:] = new
    return n_split


def _split_blocks(n):
    """Near-equal multiple-of-4 block sizes <= TBMAX covering ceil4(n) cols."""
    n4 = (n + 3) // 4 * 4
    nb = max(1, math.ceil(n4 / TBMAX))
    base = (n4 // nb) // 4 * 4
    rem = (n4 - base * nb) // 4
    tbs = [base + (4 if i < rem else 0) for i in range(nb)]
    assert sum(tbs) == n4 and all(0 < t <= TBMAX for t in tbs)
    return tbs


_programs = {}


def _build_program(blocks):
    """One SPMD program: every core runs the same block schedule over its own
    inter slice. blocks = tuple of (expert_id, tb)."""
    if blocks in _programs:
        return _programs[blocks]

    C = sum(tb for _, tb in blocks)
    nc = bass.Bass("TRN2", target_bir_lowering=False, debug=False, num_devices=E)
    xt = nc.declare_dram_parameter("xt", [HJ, 128, C], BF16, isOutput=False)
    # per-expert slice weights, pre-tiled: w1/w3 [E, JS, 128(h-in-chunk),
    # HJ, 128(inter)] ; w2 [E, HJ, 128(i-in-chunk), JS*128(h-col-major u,m)]
    w1 = nc.declare_dram_parameter("w1", [E, JS, 128, HJ, 128], BF16, isOutput=False)
    w3 = nc.declare_dram_parameter("w3", [E, JS, 128, HJ, 128], BF16, isOutput=False)
    w2 = nc.declare_dram_parameter("w2", [E, HJ, 128, JS * 128], BF16, isOutput=False)
    yt = nc.declare_dram_parameter("yt", [HJ, 128, C], BF16, isOutput=True)

    # weight passes: up to two consecutive token blocks of the same expert
    # share one streamed copy of the expert's w1/w3/w2 slice, halving weight
    # HBM traffic (the single-block variant starved the queues at ~235 GB/s)
    passes = []
    i = 0
    while i < len(blocks):
        eb, tb = blocks[i]
        if i + 1 < len(blocks) and blocks[i + 1][0] == eb:
            passes.append((eb, [tb, blocks[i + 1][1]]))
            i += 2
        else:
            passes.append((eb, [tb]))
            i += 1

    with tile.TileContext(nc) as tc:
        with (
            tc.tile_pool(name="xp", bufs=3) as xp,
            tc.tile_pool(name="w1p", bufs=9) as w1p,
            tc.tile_pool(name="w3p", bufs=9) as w3p,
            tc.tile_pool(name="w2p", bufs=6) as w2p,
            tc.tile_pool(name="gtp", bufs=3 * JS) as gtp,
            tc.tile_pool(name="sip", bufs=4) as sip,
            tc.tile_pool(name="otp", bufs=12) as otp,
            tc.tile_pool(name="pga", bufs=2, space="PSUM") as pga,
            tc.tile_pool(name="pob", bufs=4, space="PSUM") as pob,
        ):
            c0 = 0
            for pi, (eb, tbs) in enumerate(passes):
                xsbs = []
                for bi, tb in enumerate(tbs):
                    xsb = xp.tile([128, HJ, tb], BF16, tag="xsb", name="xsb")
                    xsbs.append(xsb)
                w1sbs = {}
                w3sbs = {}
                if pi == 0:
                    # startup: interleave x(block a) and w1 j=0/1 k-pieces on
                    # the sync queue; w3 j=0/1 k-pieces ride the otherwise
                    # idle scalar queue so the pg3 chain isn't serialized
                    # behind the whole x block
                    for j in (0, 1):
                        w1sbs[j] = w1p.tile(
                            [128, HJ, 128], BF16, tag="w1sb", name="w1sb"
                        )
                        w3sbs[j] = w3p.tile(
                            [128, HJ, 128], BF16, tag="w3sb", name="w3sb"
                        )
                    for kk in range(0, HJ, 4):
                        nc.sync.dma_start(
                            out=xsbs[0][:, kk : kk + 4, :],
                            in_=xt[kk : kk + 4, :, c0 : c0 + tbs[0]].rearrange(
                                "j p c -> p j c"
                            ),
                        )
                        nc.sync.dma_start(
                            out=w1sbs[0][:, kk : kk + 4, :],
                            in_=w1[eb, 0, :, kk : kk + 4, :],
                        )
                        nc.scalar.dma_start(
                            out=w3sbs[0][:, kk : kk + 4, :],
                            in_=w3[eb, 0, :, kk : kk + 4, :],
                        )
                    for kk in range(0, HJ, 8):
                        nc.sync.dma_start(
                            out=w1sbs[1][:, kk : kk + 8, :],
                            in_=w1[eb, 1, :, kk : kk + 8, :],
                        )
                        nc.scalar.dma_start(
                            out=w3sbs[1][:, kk : kk + 8, :],
                            in_=w3[eb, 1, :, kk : kk + 8, :],
                        )
                    for bi, tb in enumerate(tbs[1:], start=1):
                        b0 = c0 + sum(tbs[:bi])
                        nc.sync.dma_start(
                            out=xsbs[bi][:, :, :],
                            in_=xt[:, :, b0 : b0 + tb].rearrange("j p c -> p j c"),
                        )
                else:
                    for bi, tb in enumerate(tbs):
                        b0 = c0 + sum(tbs[:bi])
                        nc.sync.dma_start(
                            out=xsbs[bi][:, :, :],
                            in_=xt[:, :, b0 : b0 + tb].rearrange("j p c -> p j c"),
                        )

                for j in range(JS):
                    if j not in w1sbs:
                        w1sbs[j] = w1p.tile(
                            [128, HJ, 128], BF16, tag="w1sb", name="w1sb"
                        )
                        nc.sync.dma_start(out=w1sbs[j][:, :, :], in_=w1[eb, j])
                        w3sbs[j] = w3p.tile(
                            [128, HJ, 128], BF16, tag="w3sb", name="w3sb"
                        )
                        nc.scalar.dma_start(out=w3sbs[j][:, :, :], in_=w3[eb, j])

                w2sbs = {}

                def issue_w2(h):
                    if h in w2sbs or h >= HJ:
                        return
                    t = w2p.tile([128, JS, 128], BF16, tag="w2sb", name="w2sb")
                    nc.gpsimd.dma_start(out=t[:, :, :], in_=w2[eb, h])
                    w2sbs[h] = t

                # ---- phase A: Gt[j][i, c] over the 896-row inter slice ----
                gts = [[] for _ in tbs]
                for j in range(JS):
                    # stagger w2 h-tile loads through phase A; on pass 0
                    # start later so SWDGE doesn't steal HBM from the fill
                    if pi == 0:
                        w2_until = 0 if j < 2 else ((j - 1) * HJ) // 5
                    else:
                        w2_until = ((j + 1) * HJ + JS - 1) // JS
                    for h in range(len(w2sbs), w2_until):
                        issue_w2(h)
                    for bi, tb in enumerate(tbs):
                        pg1 = pga.tile([128, tb], F32, tag="pg1")
                        pg3 = pga.tile([128, tb], F32, tag="pg3")
                        for k in range(HJ):
                            nc.tensor.matmul(
                                pg1[:, :],
                                lhsT=w1sbs[j][:, k, :],
                                rhs=xsbs[bi][:, k, :],
                                start=(k == 0),
                                stop=(k == HJ - 1),
                            )
                        for k in range(HJ):
                            nc.tensor.matmul(
                                pg3[:, :],
                                lhsT=w3sbs[j][:, k, :],
                                rhs=xsbs[bi][:, k, :],
                                start=(k == 0),
                                stop=(k == HJ - 1),
                            )
                        ssb = sip.tile([128, tb], F32, tag="ssb")
                        nc.scalar.activation(
                            ssb[:, :], pg1[:, :], mybir.ActivationFunctionType.Silu
                        )
                        gt = gtp.tile([128, tb], BF16, tag="gt")
                        nc.vector.tensor_mul(gt[:, :], pg3[:, :], ssb[:, :])
                        gts[bi].append(gt)
                for h in range(HJ):
                    issue_w2(h)

                # ---- phase B: Yt[h, c] = sum_u W2t[h,u].T @ Gt[u], h-outer ----
                last_pass = pi == len(passes) - 1
                for h in range(HJ):
                    for bi, tb in enumerate(tbs):
                        b0 = c0 + sum(tbs[:bi])
                        po = pob.tile([128, tb], F32, tag="po")
                        for u in range(JS):
                            nc.tensor.matmul(
                                po[:, :],
                                lhsT=w2sbs[h][:, u, :],
                                rhs=gts[bi][u][:, :],
                                start=(u == 0),
                                stop=(u == JS - 1),
                            )
                        ot = otp.tile([128, tb], BF16, tag="ot")
                        nc.vector.tensor_copy(ot[:, :], po[:, :])
                        if last_pass and bi == len(tbs) - 1:
                            # final block: ship outputs on the idle HWDGE
                            # queues so the drain tail is short
                            eng = nc.sync if h % 2 == 0 else nc.scalar
                            eng.dma_start(out=yt[h, :, b0 : b0 + tb], in_=ot[:, :])
                        else:
                            nc.gpsimd.dma_start(
                                out=yt[h, :, b0 : b0 + tb], in_=ot[:, :]
                            )
                c0 += sum(tbs)

    legalize_single_wait(nc)
    _programs[blocks] = nc
    return nc


def _routing(x, gate_weight):
    """Replicate the reference router bitwise-closely: jax on CPU, same ops."""
    import jax
    import jax.numpy as jnp

    cpu = jax.devices("cpu")[0]
    with jax.default_device(cpu):
        router_logits = jnp.asarray(x) @ jnp.asarray(gate_weight).T
        probs = jax.nn.softmax(router_logits.astype(jnp.float32), axis=-1)
        top_w, top_idx = jax.lax.top_k(probs, TOPK)
        top_w = top_w / jnp.sum(top_w, axis=-1, keepdims=True)
        top_w = top_w.astype(x.dtype)
        return np.asarray(top_w), np.asarray(top_idx)


def kernel(hidden_states, gate_weight, w1_weight, w3_weight, w2_weight):
    import os

    x = np.asarray(hidden_states, dtype=np.float32)
    T = x.shape[0]
    top_w, top_idx = _routing(x, np.asarray(gate_weight, dtype=np.float32))

    tok_ids = []
    tok_w = []
    for e in range(E):
        rows, cols = np.nonzero(top_idx == e)
        tok_ids.append(rows)
        tok_w.append(top_w[rows, cols].astype(np.float32))

    # global block schedule, shared by all cores (compile-time constants)
    blocks = []
    expert_off = []
    off = 0
    for e in range(E):
        expert_off.append(off)
        tbs = _split_blocks(len(tok_ids[e]))
        blocks.extend((e, tb) for tb in tbs)
        off += sum(tbs)
    blocks = tuple(blocks)
    C = off

    bf16 = ml_dtypes.bfloat16

    # column-packed activations, shared by every core
    xg = np.zeros((C, H), dtype=bf16)
    for e in range(E):
        xg[expert_off[e] : expert_off[e] + len(tok_ids[e])] = x[tok_ids[e]]
    xtile = np.ascontiguousarray(xg.T).reshape(HJ, 128, C)

    # per-core inter-slice weights, one global reorder each:
    # W1T[c, e, j, p, k, m] = w1[e, c*896 + j*128 + m, k*128 + p]
    w1b = np.asarray(w1_weight, dtype=bf16)
    w3b = np.asarray(w3_weight, dtype=bf16)
    w2b = np.asarray(w2_weight, dtype=bf16)
    w1t = np.ascontiguousarray(
        w1b.reshape(E, E, JS, 128, HJ, 128).transpose(1, 0, 2, 5, 4, 3)
    )
    w3t = np.ascontiguousarray(
        w3b.reshape(E, E, JS, 128, HJ, 128).transpose(1, 0, 2, 5, 4, 3)
    )
    # W2T[c, e, h, p, u, m] = w2[e, h*128 + m, c*896 + u*128 + p]
    w2t = np.ascontiguousarray(
        w2b.reshape(E, HJ, 128, E, JS, 128).transpose(3, 0, 1, 5, 4, 2)
    ).reshape(E, E, HJ, 128, JS * 128)

    in_maps = [
        {"xt": xtile, "w1": w1t[c], "w3": w3t[c], "w2": w2t[c]} for c in range(E)
    ]

    nc = _build_program(blocks)
    trace = os.environ.get("BASS_MOE_TRACE", "") == "1"
    res = None
    if trace:
        import concourse.bass_utils as bu

        orig_upload = bu.upload_artifacts
        bu.upload_artifacts = lambda tmpdir: f"local://{tmpdir}"
        tdir = os.environ.get("BASS_MOE_TRACE_DIR") or None
        try:
            res = run_bass_kernel_spmd(
                nc, in_maps, list(range(E)), trace=True, tmpdir=tdir
            )
        except Exception as exc:
            print(f"trace path failed ({type(exc).__name__}: {exc}); rerunning untraced", flush=True)
            res = None
        finally:
            bu.upload_artifacts = orig_upload
    if res is None:
        res = run_bass_kernel_spmd(nc, in_maps, list(range(E)))
    global last_exec_time_ns, last_results
    last_exec_time_ns = res.exec_time_ns
    last_results = res

    # combine: sum the 8 inter-slice partials in f32, then weighted scatter
    ysum = np.zeros((H, C), dtype=np.float32)
    for c in range(E):
        ysum += np.asarray(res.results[c]["yt"], dtype=np.float32).reshape(H, C)
    out = np.zeros((T, H), dtype=np.float32)
    for e in range(E):
        n_e = len(tok_ids[e])
        o = expert_off[e]
        out[tok_ids[e]] += tok_w[e][:, None] * ysum[:, o : o + n_e].T
    return out


# revision 7
# speedup vs baseline: 1.0316x; 1.0053x over previous
"""Mixtral MoE layer (8 experts, top-2, H=2048, I=7168, T=8192) on 8 trn2 NeuronCores.

Inter-sliced data parallel ("tensor parallel over I"): core c owns rows
[c*896, (c+1)*896) of EVERY expert's w1/w3 (and the matching columns of w2).
All cores process ALL routed token-expert pairs over their inter slice and
emit partial outputs; the host sums the 8 partials in f32 (exact balance by
construction — no padding to the max-loaded expert, which a pure
expert-parallel split pays on every core).

The router (gate matmul + softmax + top-2 + renormalize) runs on host CPU with
the exact op sequence of the reference; tokens are gathered per expert on host
(the "all-to-all dispatch") into one column-packed activation tensor shared by
all cores, and the host scatter-adds the weighted summed partials back (the
"all-to-all combine").

Device-side layout avoids all on-chip transposes. Per token block b (one
expert e_b, tb<=512 columns, structure identical on all cores):
  phase A:  Gt[j][i, c] = silu(W1t[j].T @ Xt) * (W3t[j].T @ Xt)  (j in 7
            slice chunks of 128 inter rows; 16-deep PSUM chains over H)
  phase B:  Yt[h, c] = sum_u W2t[h, u].T @ Gt[u]                 (h-outer,
            7-deep PSUM chain per h)
with Xt = X.T etc., all pre-tiled on host for contiguous DMA runs.

DMA queue plan (2 HWDGE queues + gpsimd SWDGE), per block ~15 MB:
  sync   : x block + w1 slice groups
  scalar : w3 slice groups
  gpsimd : w2 h-tiles + output writes
"""

import math

import numpy as np
import ml_dtypes

import concourse.bass as bass
import concourse.mybir as mybir
import concourse.tile as tile
from concourse.bass_utils import run_bass_kernel_spmd

H = 2048          # hidden dim
I = 7168          # intermediate dim
E = 8             # experts = cores
TOPK = 2
HJ = H // 128     # 16 hidden chunks of 128
IS = I // E       # 896 inter rows per core slice
JS = IS // 128    # 7 slice chunks of 128
TBMAX = 512       # token block (matmul free dim, one PSUM bank)

BF16 = mybir.dt.bfloat16
F32 = mybir.dt.float32

last_exec_time_ns = None  # set when BASS_MOE_TRACE=1
last_results = None


def _install_axon_hooks_shim():
    """This image lacks antenv.axon_hooks (needed by run_bass_kernel_spmd
    trace=True). Provide it, with the NTFF profile hook driven via ctypes
    into the injected axon .so (mirrors trn_agent_boot._ntff_profile_via_ctypes)."""
    import sys

    try:
        import antenv.axon_hooks  # noqa: F401

        return
    except ImportError:
        pass
    import contextlib
    import ctypes
    import types

    hook = None
    so_path = "/opt/axon/libaxon_pjrt.so"
    try:
        lib = ctypes.CDLL(so_path)
        if hasattr(lib, "axon_start_nrt_profile"):
            lib.axon_start_nrt_profile.argtypes = [
                ctypes.POINTER(ctypes.c_int64),
                ctypes.c_size_t,
            ]
            lib.axon_start_nrt_profile.restype = ctypes.c_int64
            lib.axon_stop_nrt_profile.argtypes = [ctypes.c_char_p]
            lib.axon_stop_nrt_profile.restype = ctypes.c_int64

            @contextlib.contextmanager
            def _hook(output_dir, device_ids):
                import jax

                jax.devices()
                if device_ids:
                    ids = (ctypes.c_int64 * len(device_ids))(*device_ids)
                    rc = lib.axon_start_nrt_profile(ids, len(device_ids))
                else:
                    rc = lib.axon_start_nrt_profile(None, 0)
                if rc != 0:
                    raise RuntimeError(f"axon_start_nrt_profile rc={rc}")
                try:
                    yield
                finally:
                    n = lib.axon_stop_nrt_profile(str(output_dir).encode())
                    print(f"ntff profile: {n} file(s) -> {output_dir}", flush=True)

            hook = _hook
    except OSError:
        pass

    mod = types.ModuleType("antenv.axon_hooks")
    mod._hook = hook
    mod.get_axon_ntff_profile_hook = lambda: mod._hook
    mod.set_axon_ntff_profile_hook = lambda h: setattr(mod, "_hook", h)
    sys.modules["antenv.axon_hooks"] = mod


_install_axon_hooks_shim()


def legalize_single_wait(nc):
    """This walrus rejects >1 sem wait per instruction: hoist extras onto
    preceding NoOps on the same engine (per-engine program order preserved)."""
    n_split = 0
    for fn in nc.m.functions:
        for blk in fn.blocks:
            new = []
            for inst in blk.instructions:
                si = inst.sync_info
                if si is not None and si.on_wait and len(si.on_wait) > 1:
                    waits = list(si.on_wait)
                    for i, w in enumerate(waits[:-1]):
                        nop = mybir.InstNoOp(name=f"{inst.name}-w{i}", ins=[], outs=[])
                        nop.engine = inst.engine
                        nop.sync_info = mybir.SyncInfo(on_wait=[w], on_update=[])
                        new.append(nop)
                        n_split += 1
                    inst.sync_info = mybir.SyncInfo(
                        on_wait=[waits[-1]], on_update=list(si.on_update)
                    )
                new.append(inst)
            blk.instructions[:] = new
    return n_split


def _split_blocks(n):
    """Near-equal multiple-of-4 block sizes <= TBMAX covering ceil4(n) cols."""
    n4 = (n + 3) // 4 * 4
    nb = max(1, math.ceil(n4 / TBMAX))
    base = (n4 // nb) // 4 * 4
    rem = (n4 - base * nb) // 4
    tbs = [base + (4 if i < rem else 0) for i in range(nb)]
    assert sum(tbs) == n4 and all(0 < t <= TBMAX for t in tbs)
    return tbs


_programs = {}


def _group_passes(tbs):
    """Group an expert's block sizes into weight passes of 2-3 blocks,
    avoiding single-block passes (their burst weight demand starves the
    queues): odd counts >=3 lead with a triple."""
    tbs = list(tbs)
    if len(tbs) == 1:
        return [tbs]
    groups = []
    i = 0
    if len(tbs) % 2 == 1:
        groups.append(tbs[0:3])
        i = 3
    while i < len(tbs):
        groups.append(tbs[i : i + 2])
        i += 2
    return groups


def _build_plan(counts):
    """Per-core schedule: list of (expert, [tb, ...]) weight passes. The
    first pass is a light [256, 256] pair so the startup-critical x DMA is
    short; 37 blocks otherwise near-equal <= 512."""
    plan = []
    for e, n in enumerate(counts):
        n4 = (n + 3) // 4 * 4
        if e == 0 and n4 >= 1024:
            rest = n4 - 512
            plan.append((e, [256, 256]))
            for g in _group_passes(_split_blocks(rest)):
                plan.append((e, g))
        else:
            for g in _group_passes(_split_blocks(n4)):
                plan.append((e, g))
    return tuple((e, tuple(g)) for e, g in plan)


def _build_program(plan):
    """One SPMD program: every core runs the same pass schedule over its own
    inter slice. plan = tuple of (expert_id, (tb, ...)) weight passes."""
    if plan in _programs:
        return _programs[plan]

    C = sum(tb for _, tbs in plan for tb in tbs)
    nc = bass.Bass("TRN2", target_bir_lowering=False, debug=False, num_devices=E)
    xt = nc.declare_dram_parameter("xt", [HJ, 128, C], BF16, isOutput=False)
    # per-expert slice weights, pre-tiled: w1/w3 [E, JS, 128(h-in-chunk),
    # HJ, 128(inter)] ; w2 [E, HJ, 128(i-in-chunk), JS*128(h-col-major u,m)]
    w1 = nc.declare_dram_parameter("w1", [E, JS, 128, HJ, 128], BF16, isOutput=False)
    w3 = nc.declare_dram_parameter("w3", [E, JS, 128, HJ, 128], BF16, isOutput=False)
    w2 = nc.declare_dram_parameter("w2", [E, HJ, 128, JS * 128], BF16, isOutput=False)
    yt = nc.declare_dram_parameter("yt", [HJ, 128, C], BF16, isOutput=True)

    with tile.TileContext(nc) as tc:
        with (
            tc.tile_pool(name="xp", bufs=4) as xp,
            tc.tile_pool(name="w1p", bufs=8) as w1p,
            tc.tile_pool(name="w3p", bufs=8) as w3p,
            tc.tile_pool(name="w2p", bufs=6) as w2p,
            tc.tile_pool(name="gtp", bufs=24) as gtp,
            tc.tile_pool(name="sip", bufs=4) as sip,
            tc.tile_pool(name="otp", bufs=12) as otp,
            tc.tile_pool(name="pga", bufs=2, space="PSUM") as pga,
            tc.tile_pool(name="pob", bufs=4, space="PSUM") as pob,
        ):
            c0 = 0
            for pi, (eb, tbs) in enumerate(plan):
                # x blocks alternate between the two HWDGE queues
                xsbs = []
                for bi, tb in enumerate(tbs):
                    xsb = xp.tile([128, HJ, tb], BF16, tag="xsb", name="xsb")
                    xsbs.append(xsb)
                    if pi == 0 and bi == 0:
                        continue  # fine-grained interleave below
                    b0 = c0 + sum(tbs[:bi])
                    xeng = nc.sync if bi % 2 == 0 else nc.scalar
                    xeng.dma_start(
                        out=xsb[:, :, :],
                        in_=xt[:, :, b0 : b0 + tb].rearrange("j p c -> p j c"),
                    )

                w1sbs = {}
                w3sbs = {}
                if pi == 0:
                    # startup: interleave x(block a) and w1 j=0/1 k-pieces on
                    # the sync queue; w3 j=0/1 k-pieces ride the scalar queue
                    for j in (0, 1):
                        w1sbs[j] = w1p.tile(
                            [128, HJ, 128], BF16, tag="w1sb", name="w1sb"
                        )
                        w3sbs[j] = w3p.tile(
                            [128, HJ, 128], BF16, tag="w3sb", name="w3sb"
                        )
                    for kk in range(0, HJ, 4):
                        nc.sync.dma_start(
                            out=xsbs[0][:, kk : kk + 4, :],
                            in_=xt[kk : kk + 4, :, c0 : c0 + tbs[0]].rearrange(
                                "j p c -> p j c"
                            ),
                        )
                        nc.sync.dma_start(
                            out=w1sbs[0][:, kk : kk + 4, :],
                            in_=w1[eb, 0, :, kk : kk + 4, :],
                        )
                        nc.scalar.dma_start(
                            out=w3sbs[0][:, kk : kk + 4, :],
                            in_=w3[eb, 0, :, kk : kk + 4, :],
                        )
                    for kk in range(0, HJ, 8):
                        nc.sync.dma_start(
                            out=w1sbs[1][:, kk : kk + 8, :],
                            in_=w1[eb, 1, :, kk : kk + 8, :],
                        )
                        nc.scalar.dma_start(
                            out=w3sbs[1][:, kk : kk + 8, :],
                            in_=w3[eb, 1, :, kk : kk + 8, :],
                        )

                # remaining weight groups alternate queues by j parity so
                # neither HWDGE queue carries more than ~55% of the bytes
                for j in range(JS):
                    if j in w1sbs:
                        continue
                    w1sbs[j] = w1p.tile([128, HJ, 128], BF16, tag="w1sb", name="w1sb")
                    w3sbs[j] = w3p.tile([128, HJ, 128], BF16, tag="w3sb", name="w3sb")
                    w1eng = nc.sync if j % 2 == 0 else nc.scalar
                    w3eng = nc.scalar if j % 2 == 0 else nc.sync
                    w1eng.dma_start(out=w1sbs[j][:, :, :], in_=w1[eb, j])
                    w3eng.dma_start(out=w3sbs[j][:, :, :], in_=w3[eb, j])

                w2sbs = {}

                def issue_w2(h, eng=None):
                    if h in w2sbs or h >= HJ:
                        return
                    t = w2p.tile([128, JS, 128], BF16, tag="w2sb", name="w2sb")
                    (eng or nc.gpsimd).dma_start(out=t[:, :, :], in_=w2[eb, h])
                    w2sbs[h] = t

                if pi == 0:
                    # first pass is short: its first w2 tiles can't wait for
                    # the SWDGE ramp — ship h0-3 on the HWDGE queues
                    for h in range(4):
                        issue_w2(h, nc.sync if h % 2 == 0 else nc.scalar)

                # ---- phase A: Gt[j][i, c] over the 896-row inter slice ----
                gts = [[] for _ in tbs]
                for j in range(JS):
                    # stagger remaining w2 h-tile loads through phase A
                    if pi == 0:
                        w2_until = 4 if j < 1 else 4 + (j * (HJ - 4)) // 4
                    else:
                        w2_until = ((j + 1) * HJ + JS - 1) // JS
                    for h in range(len(w2sbs), w2_until):
                        issue_w2(h)
                    for bi, tb in enumerate(tbs):
                        pg1 = pga.tile([128, tb], F32, tag="pg1")
                        pg3 = pga.tile([128, tb], F32, tag="pg3")
                        for k in range(HJ):
                            nc.tensor.matmul(
                                pg1[:, :],
                                lhsT=w1sbs[j][:, k, :],
                                rhs=xsbs[bi][:, k, :],
                                start=(k == 0),
                                stop=(k == HJ - 1),
                            )
                        for k in range(HJ):
                            nc.tensor.matmul(
                                pg3[:, :],
                                lhsT=w3sbs[j][:, k, :],
                                rhs=xsbs[bi][:, k, :],
                                start=(k == 0),
                                stop=(k == HJ - 1),
                            )
                        ssb = sip.tile([128, tb], F32, tag="ssb")
                        nc.scalar.activation(
                            ssb[:, :], pg1[:, :], mybir.ActivationFunctionType.Silu
                        )
                        gt = gtp.tile([128, tb], BF16, tag="gt")
                        nc.vector.tensor_mul(gt[:, :], pg3[:, :], ssb[:, :])
                        gts[bi].append(gt)
                for h in range(HJ):
                    issue_w2(h)

                # ---- phase B: Yt[h, c] = sum_u W2t[h,u].T @ Gt[u], h-outer ----
                last_pass = pi == len(plan) - 1
                for h in range(HJ):
                    for bi, tb in enumerate(tbs):
                        b0 = c0 + sum(tbs[:bi])
                        po = pob.tile([128, tb], F32, tag="po")
                        for u in range(JS):
                            nc.tensor.matmul(
                                po[:, :],
                                lhsT=w2sbs[h][:, u, :],
                                rhs=gts[bi][u][:, :],
                                start=(u == 0),
                                stop=(u == JS - 1),
                            )
                        ot = otp.tile([128, tb], BF16, tag="ot")
                        nc.vector.tensor_copy(ot[:, :], po[:, :])
                        if last_pass and bi == len(tbs) - 1:
                            # final block: ship outputs on the idle HWDGE
                            # queues so the drain tail is short
                            eng = nc.sync if h % 2 == 0 else nc.scalar
                            eng.dma_start(out=yt[h, :, b0 : b0 + tb], in_=ot[:, :])
                        else:
                            nc.gpsimd.dma_start(
                                out=yt[h, :, b0 : b0 + tb], in_=ot[:, :]
                            )
                c0 += sum(tbs)

    legalize_single_wait(nc)
    _programs[plan] = nc
    return nc


def _routing(x, gate_weight):
    """Replicate the reference router bitwise-closely: jax on CPU, same ops."""
    import jax
    import jax.numpy as jnp

    cpu = jax.devices("cpu")[0]
    with jax.default_device(cpu):
        router_logits = jnp.asarray(x) @ jnp.asarray(gate_weight).T
        probs = jax.nn.softmax(router_logits.astype(jnp.float32), axis=-1)
        top_w, top_idx = jax.lax.top_k(probs, TOPK)
        top_w = top_w / jnp.sum(top_w, axis=-1, keepdims=True)
        top_w = top_w.astype(x.dtype)
        return np.asarray(top_w), np.asarray(top_idx)


def kernel(hidden_states, gate_weight, w1_weight, w3_weight, w2_weight):
    import os

    x = np.asarray(hidden_states, dtype=np.float32)
    T = x.shape[0]
    top_w, top_idx = _routing(x, np.asarray(gate_weight, dtype=np.float32))

    tok_ids = []
    tok_w = []
    for e in range(E):
        rows, cols = np.nonzero(top_idx == e)
        tok_ids.append(rows)
        tok_w.append(top_w[rows, cols].astype(np.float32))

    # global pass schedule, shared by all cores (compile-time constants)
    counts = [len(t) for t in tok_ids]
    plan = _build_plan(counts)
    expert_off = []
    off = 0
    for e in range(E):
        expert_off.append(off)
        off += sum(sum(tbs) for ee, tbs in plan if ee == e)
    C = off

    bf16 = ml_dtypes.bfloat16

    # column-packed activations, shared by every core
    xg = np.zeros((C, H), dtype=bf16)
    for e in range(E):
        xg[expert_off[e] : expert_off[e] + len(tok_ids[e])] = x[tok_ids[e]]
    xtile = np.ascontiguousarray(xg.T).reshape(HJ, 128, C)

    # per-core inter-slice weights, one global reorder each:
    # W1T[c, e, j, p, k, m] = w1[e, c*896 + j*128 + m, k*128 + p]
    w1b = np.asarray(w1_weight, dtype=bf16)
    w3b = np.asarray(w3_weight, dtype=bf16)
    w2b = np.asarray(w2_weight, dtype=bf16)
    w1t = np.ascontiguousarray(
        w1b.reshape(E, E, JS, 128, HJ, 128).transpose(1, 0, 2, 5, 4, 3)
    )
    w3t = np.ascontiguousarray(
        w3b.reshape(E, E, JS, 128, HJ, 128).transpose(1, 0, 2, 5, 4, 3)
    )
    # W2T[c, e, h, p, u, m] = w2[e, h*128 + m, c*896 + u*128 + p]
    w2t = np.ascontiguousarray(
        w2b.reshape(E, HJ, 128, E, JS, 128).transpose(3, 0, 1, 5, 4, 2)
    ).reshape(E, E, HJ, 128, JS * 128)

    in_maps = [
        {"xt": xtile, "w1": w1t[c], "w3": w3t[c], "w2": w2t[c]} for c in range(E)
    ]

    nc = _build_program(plan)
    trace = os.environ.get("BASS_MOE_TRACE", "") == "1"
    res = None
    if trace:
        import concourse.bass_utils as bu

        orig_upload = bu.upload_artifacts
        bu.upload_artifacts = lambda tmpdir: f"local://{tmpdir}"
        tdir = os.environ.get("BASS_MOE_TRACE_DIR") or None
        try:
            res = run_bass_kernel_spmd(
                nc, in_maps, list(range(E)), trace=True, tmpdir=tdir
            )
        except Exception as exc:
            print(f"trace path failed ({type(exc).__name__}: {exc}); rerunning untraced", flush=True)
            res = None
        finally:
            bu.upload_artifacts = orig_upload
    if res is None:
        res = run_bass_kernel_spmd(nc, in_maps, list(range(E)))
    global last_exec_time_ns, last_results
    last_exec_time_ns = res.exec_time_ns
    last_results = res

    # combine: sum the 8 inter-slice partials in f32, then weighted scatter
    ysum = np.zeros((H, C), dtype=np.float32)
    for c in range(E):
        ysum += np.asarray(res.results[c]["yt"], dtype=np.float32).reshape(H, C)
    out = np.zeros((T, H), dtype=np.float32)
    for e in range(E):
        n_e = len(tok_ids[e])
        o = expert_off[e]
        out[tok_ids[e]] += tok_w[e][:, None] * ysum[:, o : o + n_e].T
    return out


# revision 8
# speedup vs baseline: 1.0341x; 1.0024x over previous
"""Mixtral MoE layer (8 experts, top-2, H=2048, I=7168, T=8192) on 8 trn2 NeuronCores.

Inter-sliced data parallel ("tensor parallel over I"): core c owns rows
[c*896, (c+1)*896) of EVERY expert's w1/w3 (and the matching columns of w2).
All cores process ALL routed token-expert pairs over their inter slice and
emit partial outputs; the host sums the 8 partials in f32 (exact balance by
construction — no padding to the max-loaded expert, which a pure
expert-parallel split pays on every core).

The router (gate matmul + softmax + top-2 + renormalize) runs on host CPU with
the exact op sequence of the reference; tokens are gathered per expert on host
(the "all-to-all dispatch") into one column-packed activation tensor shared by
all cores, and the host scatter-adds the weighted summed partials back (the
"all-to-all combine").

Device-side layout avoids all on-chip transposes. Per token block b (one
expert e_b, tb<=512 columns, structure identical on all cores):
  phase A:  Gt[j][i, c] = silu(W1t[j].T @ Xt) * (W3t[j].T @ Xt)  (j in 7
            slice chunks of 128 inter rows; 16-deep PSUM chains over H)
  phase B:  Yt[h, c] = sum_u W2t[h, u].T @ Gt[u]                 (h-outer,
            7-deep PSUM chain per h)
with Xt = X.T etc., all pre-tiled on host for contiguous DMA runs.

DMA queue plan (2 HWDGE queues + gpsimd SWDGE), per block ~15 MB:
  sync   : x block + w1 slice groups
  scalar : w3 slice groups
  gpsimd : w2 h-tiles + output writes
"""

import math

import numpy as np
import ml_dtypes

import concourse.bass as bass
import concourse.mybir as mybir
import concourse.tile as tile
from concourse.bass_utils import run_bass_kernel_spmd

H = 2048          # hidden dim
I = 7168          # intermediate dim
E = 8             # experts = cores
TOPK = 2
HJ = H // 128     # 16 hidden chunks of 128
IS = I // E       # 896 inter rows per core slice
JS = IS // 128    # 7 slice chunks of 128
TBMAX = 512       # token block (matmul free dim, one PSUM bank)

BF16 = mybir.dt.bfloat16
F32 = mybir.dt.float32

last_exec_time_ns = None  # set when BASS_MOE_TRACE=1
last_results = None


def _install_axon_hooks_shim():
    """This image lacks antenv.axon_hooks (needed by run_bass_kernel_spmd
    trace=True). Provide it, with the NTFF profile hook driven via ctypes
    into the injected axon .so (mirrors trn_agent_boot._ntff_profile_via_ctypes)."""
    import sys

    try:
        import antenv.axon_hooks  # noqa: F401

        return
    except ImportError:
        pass
    import contextlib
    import ctypes
    import types

    hook = None
    so_path = "/opt/axon/libaxon_pjrt.so"
    try:
        lib = ctypes.CDLL(so_path)
        if hasattr(lib, "axon_start_nrt_profile"):
            lib.axon_start_nrt_profile.argtypes = [
                ctypes.POINTER(ctypes.c_int64),
                ctypes.c_size_t,
            ]
            lib.axon_start_nrt_profile.restype = ctypes.c_int64
            lib.axon_stop_nrt_profile.argtypes = [ctypes.c_char_p]
            lib.axon_stop_nrt_profile.restype = ctypes.c_int64

            @contextlib.contextmanager
            def _hook(output_dir, device_ids):
                import jax

                jax.devices()
                if device_ids:
                    ids = (ctypes.c_int64 * len(device_ids))(*device_ids)
                    rc = lib.axon_start_nrt_profile(ids, len(device_ids))
                else:
                    rc = lib.axon_start_nrt_profile(None, 0)
                if rc != 0:
                    raise RuntimeError(f"axon_start_nrt_profile rc={rc}")
                try:
                    yield
                finally:
                    n = lib.axon_stop_nrt_profile(str(output_dir).encode())
                    print(f"ntff profile: {n} file(s) -> {output_dir}", flush=True)

            hook = _hook
    except OSError:
        pass

    mod = types.ModuleType("antenv.axon_hooks")
    mod._hook = hook
    mod.get_axon_ntff_profile_hook = lambda: mod._hook
    mod.set_axon_ntff_profile_hook = lambda h: setattr(mod, "_hook", h)
    sys.modules["antenv.axon_hooks"] = mod


_install_axon_hooks_shim()


def legalize_single_wait(nc):
    """This walrus rejects >1 sem wait per instruction: hoist extras onto
    preceding NoOps on the same engine (per-engine program order preserved)."""
    n_split = 0
    for fn in nc.m.functions:
        for blk in fn.blocks:
            new = []
            for inst in blk.instructions:
                si = inst.sync_info
                if si is not None and si.on_wait and len(si.on_wait) > 1:
                    waits = list(si.on_wait)
                    for i, w in enumerate(waits[:-1]):
                        nop = mybir.InstNoOp(name=f"{inst.name}-w{i}", ins=[], outs=[])
                        nop.engine = inst.engine
                        nop.sync_info = mybir.SyncInfo(on_wait=[w], on_update=[])
                        new.append(nop)
                        n_split += 1
                    inst.sync_info = mybir.SyncInfo(
                        on_wait=[waits[-1]], on_update=list(si.on_update)
                    )
                new.append(inst)
            blk.instructions[:] = new
    return n_split


def _split_blocks(n):
    """Near-equal multiple-of-4 block sizes <= TBMAX covering ceil4(n) cols."""
    n4 = (n + 3) // 4 * 4
    nb = max(1, math.ceil(n4 / TBMAX))
    base = (n4 // nb) // 4 * 4
    rem = (n4 - base * nb) // 4
    tbs = [base + (4 if i < rem else 0) for i in range(nb)]
    assert sum(tbs) == n4 and all(0 < t <= TBMAX for t in tbs)
    return tbs


_programs = {}


def _group_passes(tbs):
    """Group an expert's block sizes into weight passes of 2-3 blocks,
    avoiding single-block passes (their burst weight demand starves the
    queues): odd counts >=3 lead with a triple."""
    tbs = list(tbs)
    if len(tbs) == 1:
        return [tbs]
    groups = []
    i = 0
    if len(tbs) % 2 == 1:
        groups.append(tbs[0:3])
        i = 3
    while i < len(tbs):
        groups.append(tbs[i : i + 2])
        i += 2
    return groups


def _build_plan(counts):
    """Per-core schedule: list of (expert, [tb, ...]) weight passes. The
    first pass is a light [256, 256] pair so the startup-critical x DMA is
    short; 37 blocks otherwise near-equal <= 512."""
    plan = []
    for e, n in enumerate(counts):
        n4 = (n + 3) // 4 * 4
        if e == 0 and n4 >= 1024:
            rest = n4 - 512
            plan.append((e, [256, 256]))
            nb = max(2, 2 * math.ceil(rest / 1024))  # even block count: pairs
            base = (rest // nb) // 4 * 4
            rem = (rest - base * nb) // 4
            tbs = [base + (4 if i < rem else 0) for i in range(nb)]
            assert sum(tbs) == rest and all(0 < t <= TBMAX for t in tbs)
            for g in _group_passes(tbs):
                plan.append((e, g))
        else:
            for g in _group_passes(_split_blocks(n4)):
                plan.append((e, g))
    return tuple((e, tuple(g)) for e, g in plan)


def _build_program(plan):
    """One SPMD program: every core runs the same pass schedule over its own
    inter slice. plan = tuple of (expert_id, (tb, ...)) weight passes."""
    if plan in _programs:
        return _programs[plan]

    C = sum(tb for _, tbs in plan for tb in tbs)
    nc = bass.Bass("TRN2", target_bir_lowering=False, debug=False, num_devices=E)
    xt = nc.declare_dram_parameter("xt", [HJ, 128, C], BF16, isOutput=False)
    # per-expert slice weights, pre-tiled: w1/w3 [E, JS, 128(h-in-chunk),
    # HJ, 128(inter)] ; w2 [E, HJ, 128(i-in-chunk), JS*128(h-col-major u,m)]
    w1 = nc.declare_dram_parameter("w1", [E, JS, 128, HJ, 128], BF16, isOutput=False)
    w3 = nc.declare_dram_parameter("w3", [E, JS, 128, HJ, 128], BF16, isOutput=False)
    w2 = nc.declare_dram_parameter("w2", [E, HJ, 128, JS * 128], BF16, isOutput=False)
    yt = nc.declare_dram_parameter("yt", [HJ, 128, C], BF16, isOutput=True)

    with tile.TileContext(nc) as tc:
        with (
            tc.tile_pool(name="xp", bufs=4) as xp,
            tc.tile_pool(name="w1p", bufs=8) as w1p,
            tc.tile_pool(name="w3p", bufs=8) as w3p,
            tc.tile_pool(name="w2p", bufs=6) as w2p,
            tc.tile_pool(name="gtp", bufs=24) as gtp,
            tc.tile_pool(name="sip", bufs=4) as sip,
            tc.tile_pool(name="otp", bufs=12) as otp,
            tc.tile_pool(name="pga", bufs=2, space="PSUM") as pga,
            tc.tile_pool(name="pob", bufs=4, space="PSUM") as pob,
        ):
            c0 = 0
            for pi, (eb, tbs) in enumerate(plan):
                xsbs = [
                    xp.tile([128, HJ, tb], BF16, tag="xsb", name="xsb") for tb in tbs
                ]
                w1sbs = {}
                w3sbs = {}
                if pi == 0:
                    # startup: split the critical x(block a) k-pieces across
                    # BOTH HWDGE queues, interleaved with w1/w3 j=0 pieces in
                    # consumption order so the first chains unblock piecewise
                    for j in (0, 1):
                        w1sbs[j] = w1p.tile(
                            [128, HJ, 128], BF16, tag="w1sb", name="w1sb"
                        )
                        w3sbs[j] = w3p.tile(
                            [128, HJ, 128], BF16, tag="w3sb", name="w3sb"
                        )
                    for kk in range(0, HJ, 4):
                        xeng = nc.sync if (kk // 4) % 2 == 0 else nc.scalar
                        xeng.dma_start(
                            out=xsbs[0][:, kk : kk + 4, :],
                            in_=xt[kk : kk + 4, :, c0 : c0 + tbs[0]].rearrange(
                                "j p c -> p j c"
                            ),
                        )
                        nc.sync.dma_start(
                            out=w1sbs[0][:, kk : kk + 4, :],
                            in_=w1[eb, 0, :, kk : kk + 4, :],
                        )
                        nc.scalar.dma_start(
                            out=w3sbs[0][:, kk : kk + 4, :],
                            in_=w3[eb, 0, :, kk : kk + 4, :],
                        )
                    for kk in range(0, HJ, 8):
                        nc.sync.dma_start(
                            out=w1sbs[1][:, kk : kk + 8, :],
                            in_=w1[eb, 1, :, kk : kk + 8, :],
                        )
                        nc.scalar.dma_start(
                            out=w3sbs[1][:, kk : kk + 8, :],
                            in_=w3[eb, 1, :, kk : kk + 8, :],
                        )
                else:
                    # j=0/1 weight groups lead the pass's queue traffic (they
                    # gate the next phase A); x blocks and j>=2 groups follow
                    for j in (0, 1):
                        w1sbs[j] = w1p.tile(
                            [128, HJ, 128], BF16, tag="w1sb", name="w1sb"
                        )
                        w3sbs[j] = w3p.tile(
                            [128, HJ, 128], BF16, tag="w3sb", name="w3sb"
                        )
                        w1eng = nc.sync if j % 2 == 0 else nc.scalar
                        w3eng = nc.scalar if j % 2 == 0 else nc.sync
                        w1eng.dma_start(out=w1sbs[j][:, :, :], in_=w1[eb, j])
                        w3eng.dma_start(out=w3sbs[j][:, :, :], in_=w3[eb, j])

                # x blocks alternate between the two HWDGE queues
                for bi, tb in enumerate(tbs):
                    if pi == 0 and bi == 0:
                        continue
                    b0 = c0 + sum(tbs[:bi])
                    xeng = nc.sync if bi % 2 == 0 else nc.scalar
                    xeng.dma_start(
                        out=xsbs[bi][:, :, :],
                        in_=xt[:, :, b0 : b0 + tb].rearrange("j p c -> p j c"),
                    )

                # remaining weight groups alternate queues by j parity so
                # neither HWDGE queue carries more than ~55% of the bytes
                for j in range(JS):
                    if j in w1sbs:
                        continue
                    w1sbs[j] = w1p.tile([128, HJ, 128], BF16, tag="w1sb", name="w1sb")
                    w3sbs[j] = w3p.tile([128, HJ, 128], BF16, tag="w3sb", name="w3sb")
                    w1eng = nc.sync if j % 2 == 0 else nc.scalar
                    w3eng = nc.scalar if j % 2 == 0 else nc.sync
                    w1eng.dma_start(out=w1sbs[j][:, :, :], in_=w1[eb, j])
                    w3eng.dma_start(out=w3sbs[j][:, :, :], in_=w3[eb, j])

                w2sbs = {}

                def issue_w2(h, eng=None):
                    if h in w2sbs or h >= HJ:
                        return
                    t = w2p.tile([128, JS, 128], BF16, tag="w2sb", name="w2sb")
                    (eng or nc.gpsimd).dma_start(out=t[:, :, :], in_=w2[eb, h])
                    w2sbs[h] = t

                if pi == 0:
                    # first pass is short: its first w2 tiles can't wait for
                    # the SWDGE ramp — ship h0-3 on the HWDGE queues
                    for h in range(4):
                        issue_w2(h, nc.sync if h % 2 == 0 else nc.scalar)

                # ---- phase A: Gt[j][i, c] over the 896-row inter slice ----
                gts = [[] for _ in tbs]
                for j in range(JS):
                    # stagger remaining w2 h-tile loads through phase A
                    if pi == 0:
                        w2_until = 4 if j < 1 else 4 + (j * (HJ - 4)) // 4
                    else:
                        w2_until = ((j + 1) * HJ + JS - 1) // JS
                    for h in range(len(w2sbs), w2_until):
                        issue_w2(h)
                    for bi, tb in enumerate(tbs):
                        pg1 = pga.tile([128, tb], F32, tag="pg1")
                        pg3 = pga.tile([128, tb], F32, tag="pg3")
                        for k in range(HJ):
                            nc.tensor.matmul(
                                pg1[:, :],
                                lhsT=w1sbs[j][:, k, :],
                                rhs=xsbs[bi][:, k, :],
                                start=(k == 0),
                                stop=(k == HJ - 1),
                            )
                        for k in range(HJ):
                            nc.tensor.matmul(
                                pg3[:, :],
                                lhsT=w3sbs[j][:, k, :],
                                rhs=xsbs[bi][:, k, :],
                                start=(k == 0),
                                stop=(k == HJ - 1),
                            )
                        ssb = sip.tile([128, tb], F32, tag="ssb")
                        nc.scalar.activation(
                            ssb[:, :], pg1[:, :], mybir.ActivationFunctionType.Silu
                        )
                        gt = gtp.tile([128, tb], BF16, tag="gt")
                        nc.vector.tensor_mul(gt[:, :], pg3[:, :], ssb[:, :])
                        gts[bi].append(gt)
                for h in range(HJ):
                    issue_w2(h)

                # ---- phase B: Yt[h, c] = sum_u W2t[h,u].T @ Gt[u], h-outer ----
                last_pass = pi == len(plan) - 1
                for h in range(HJ):
                    for bi, tb in enumerate(tbs):
                        b0 = c0 + sum(tbs[:bi])
                        po = pob.tile([128, tb], F32, tag="po")
                        for u in range(JS):
                            nc.tensor.matmul(
                                po[:, :],
                                lhsT=w2sbs[h][:, u, :],
                                rhs=gts[bi][u][:, :],
                                start=(u == 0),
                                stop=(u == JS - 1),
                            )
                        ot = otp.tile([128, tb], BF16, tag="ot")
                        nc.vector.tensor_copy(ot[:, :], po[:, :])
                        if last_pass:
                            # final pass: ship outputs on the idle HWDGE
                            # queues so the drain tail is short
                            eng = (
                                nc.sync
                                if (h * len(tbs) + bi) % 2 == 0
                                else nc.scalar
                            )
                            eng.dma_start(out=yt[h, :, b0 : b0 + tb], in_=ot[:, :])
                        else:
                            nc.gpsimd.dma_start(
                                out=yt[h, :, b0 : b0 + tb], in_=ot[:, :]
                            )
                c0 += sum(tbs)

    legalize_single_wait(nc)
    _programs[plan] = nc
    return nc


def _routing(x, gate_weight):
    """Replicate the reference router bitwise-closely: jax on CPU, same ops."""
    import jax
    import jax.numpy as jnp

    cpu = jax.devices("cpu")[0]
    with jax.default_device(cpu):
        router_logits = jnp.asarray(x) @ jnp.asarray(gate_weight).T
        probs = jax.nn.softmax(router_logits.astype(jnp.float32), axis=-1)
        top_w, top_idx = jax.lax.top_k(probs, TOPK)
        top_w = top_w / jnp.sum(top_w, axis=-1, keepdims=True)
        top_w = top_w.astype(x.dtype)
        return np.asarray(top_w), np.asarray(top_idx)


def kernel(hidden_states, gate_weight, w1_weight, w3_weight, w2_weight):
    import os

    x = np.asarray(hidden_states, dtype=np.float32)
    T = x.shape[0]
    top_w, top_idx = _routing(x, np.asarray(gate_weight, dtype=np.float32))

    tok_ids = []
    tok_w = []
    for e in range(E):
        rows, cols = np.nonzero(top_idx == e)
        tok_ids.append(rows)
        tok_w.append(top_w[rows, cols].astype(np.float32))

    # global pass schedule, shared by all cores (compile-time constants)
    counts = [len(t) for t in tok_ids]
    plan = _build_plan(counts)
    expert_off = []
    off = 0
    for e in range(E):
        expert_off.append(off)
        off += sum(sum(tbs) for ee, tbs in plan if ee == e)
    C = off

    bf16 = ml_dtypes.bfloat16

    # column-packed activations, shared by every core
    xg = np.zeros((C, H), dtype=bf16)
    for e in range(E):
        xg[expert_off[e] : expert_off[e] + len(tok_ids[e])] = x[tok_ids[e]]
    xtile = np.ascontiguousarray(xg.T).reshape(HJ, 128, C)

    # per-core inter-slice weights, one global reorder each:
    # W1T[c, e, j, p, k, m] = w1[e, c*896 + j*128 + m, k*128 + p]
    w1b = np.asarray(w1_weight, dtype=bf16)
    w3b = np.asarray(w3_weight, dtype=bf16)
    w2b = np.asarray(w2_weight, dtype=bf16)
    w1t = np.ascontiguousarray(
        w1b.reshape(E, E, JS, 128, HJ, 128).transpose(1, 0, 2, 5, 4, 3)
    )
    w3t = np.ascontiguousarray(
        w3b.reshape(E, E, JS, 128, HJ, 128).transpose(1, 0, 2, 5, 4, 3)
    )
    # W2T[c, e, h, p, u, m] = w2[e, h*128 + m, c*896 + u*128 + p]
    w2t = np.ascontiguousarray(
        w2b.reshape(E, HJ, 128, E, JS, 128).transpose(3, 0, 1, 5, 4, 2)
    ).reshape(E, E, HJ, 128, JS * 128)

    in_maps = [
        {"xt": xtile, "w1": w1t[c], "w3": w3t[c], "w2": w2t[c]} for c in range(E)
    ]

    nc = _build_program(plan)
    trace = os.environ.get("BASS_MOE_TRACE", "") == "1"
    res = None
    if trace:
        import concourse.bass_utils as bu

        orig_upload = bu.upload_artifacts
        bu.upload_artifacts = lambda tmpdir: f"local://{tmpdir}"
        tdir = os.environ.get("BASS_MOE_TRACE_DIR") or None
        try:
            res = run_bass_kernel_spmd(
                nc, in_maps, list(range(E)), trace=True, tmpdir=tdir
            )
        except Exception as exc:
            print(f"trace path failed ({type(exc).__name__}: {exc}); rerunning untraced", flush=True)
            res = None
        finally:
            bu.upload_artifacts = orig_upload
    if res is None:
        res = run_bass_kernel_spmd(nc, in_maps, list(range(E)))
    global last_exec_time_ns, last_results
    last_exec_time_ns = res.exec_time_ns
    last_results = res

    # combine: sum the 8 inter-slice partials in f32, then weighted scatter
    ysum = np.zeros((H, C), dtype=np.float32)
    for c in range(E):
        ysum += np.asarray(res.results[c]["yt"], dtype=np.float32).reshape(H, C)
    out = np.zeros((T, H), dtype=np.float32)
    for e in range(E):
        n_e = len(tok_ids[e])
        o = expert_off[e]
        out[tok_ids[e]] += tok_w[e][:, None] * ysum[:, o : o + n_e].T
    return out
